# revision 9
# baseline (speedup 1.0000x reference)
"""Causal self-attention kernel for 8 Trainium2 NeuronCores.

Problem: B=4, T=2048, C=1024, H=16 heads, D=64 (fp32).
  qkv = x @ w_qkv + b_qkv ; causal softmax attention ; y @ w_proj + b_proj

Sharding: DP over batch (4) x TP over heads (2) = 8 cores.
Core c handles batch b=c//2 and heads h0=(c%2)*8 .. h0+7.
Each core computes a partial projection output (its 8 heads' contribution);
the host sums the two TP partials per batch and adds b_proj.

Device-side dataflow per core:
  phase 1 (per 512-token chunk n, f32r matmuls):
    qT/kT [feat, tok] = W_qk^T x^T   -> stored bf16 in SBUF
    v     [tok, feat] = x^T^T W_v    -> stored bf16 (v_aug, with ones column)
  attention (per q-group qg=n, per head-PAIR, bf16 matmuls):
    Heads 2j (partitions 0:64) and 2j+1 (partitions 64:128) are processed
    together; their ST matmuls are issued back-to-back so the PE runs them
    CONCURRENTLY in different row-groups (tile_position row packing).
    ST[k,q] = k_tile^T @ q^T_chunk  (k-tile pairs share a 2-bank PSUM tile)
    PT = exp(ST - 4.0) (ACT, bf16 out); diagonal pairs multiplied by 0/1 masks
    oT_aug[65,q] += v_aug[k,65]^T @ PT, split into two 64-row halves that the
    PE also runs concurrently (row packing); ones column gives softmax denom.
    PV for pair i is issued after ST/exp of pair i+1 (software pipelining) so
    the PE never stalls on the ACT exp.
  proj (per q-group): out[tok, C] += oT_tiles^T @ w_proj_tiles (bf16)
"""

import numpy as np

B, T, C = 4, 2048, 1024
H, D = 16, 64
NCORES = 8
HC = H // 2  # heads per core (TP=2)
CEXP = 4.0  # constant softmax offset (scores are in [-4, 4] for this problem)

TN = 512  # token chunk
NCHUNK = T // TN  # 4
KT_C = C // 128  # 8 contraction tiles for C
NQKM = C // 128  # 8 m-tiles for the qk matmul output (1024 feats)
NVSUB = TN // 128  # 4 v sub-tiles per chunk
NKT = T // 128  # 16 k-token tiles
KT_P = (HC * D) // 128  # 4 contraction tiles for proj (512 feats)

_CACHE = {}
PACK_ST = True  # issue head-pair STs adjacently (PE row-group concurrency)
BF16 = True  # bf16 matmul inputs everywhere (2 cols/cycle streaming + LDW
#              prefetch into the background weight buffer; f32r matmuls are
#              self-loading and stall ~170ns per MM on the weight load)


def _build_program(reps=1):
    # reps>1 repeats the whole kernel body inside one program (timing only:
    # the slope between rep counts isolates HW exec time from RPC overhead).
    import concourse.mybir as mybir
    import concourse.tile as tile
    from concourse import bacc

    f32 = mybir.dt.float32
    f32r = mybir.dt.float32r
    bf16 = mybir.dt.bfloat16
    mmdt = bf16 if BF16 else f32r
    xdt = bf16 if BF16 else f32

    nc = bacc.Bacc("TRN2", target_bir_lowering=False, debug=False)

    xT = nc.dram_tensor("xT", [C, T], xdt, kind="ExternalInput").ap()
    wqk = nc.dram_tensor("wqk", [C, 2 * HC * D], xdt, kind="ExternalInput").ap()
    wv = nc.dram_tensor("wv", [C, HC * D], xdt, kind="ExternalInput").ap()
    wproj = nc.dram_tensor("wproj", [HC * D, C], bf16, kind="ExternalInput").ap()
    bqk = nc.dram_tensor("bqk", [2 * HC * D], f32, kind="ExternalInput").ap()
    bv = nc.dram_tensor("bv", [HC * D], f32, kind="ExternalInput").ap()
    out = nc.dram_tensor("out", [T, C], f32, kind="ExternalOutput").ap()

    xT_r = xT.rearrange("(ko p) t -> p ko t", p=128)  # [128, 8, 2048]
    wqk_r = wqk.rearrange("(ko p) f -> p ko f", p=128)  # [128, 8, 1024]
    wv_r = wv.rearrange("(ko p) f -> p ko f", p=128)  # [128, 8, 512]
    wproj_r = wproj.rearrange("(ko p) f -> p ko f", p=128)  # [128, 4, 1024]
    bqk_r = bqk.rearrange("(m p) -> p m", p=128)  # [128, 8]

    Exp = mybir.ActivationFunctionType.Exp
    Identity = mybir.ActivationFunctionType.Identity

    with tile.TileContext(nc) as tc:
        with (
            tc.tile_pool(name="pers", bufs=1) as pers,
            tc.tile_pool(name="xc", bufs=2) as xcp,
            tc.tile_pool(name="qtc", bufs=2) as qtcp,
            tc.tile_pool(name="ptp", bufs=4) as ptp,
            tc.tile_pool(name="otc", bufs=2) as otcp,
            tc.tile_pool(name="outp", bufs=2) as outp,
            tc.tile_pool(name="rcp", bufs=2) as rcp,
            tc.tile_pool(name="rcbp", bufs=2) as rcbp,
            tc.tile_pool(name="ps_qv", bufs=2, space="PSUM") as ps_qv,
            tc.tile_pool(name="ps_st", bufs=2, space="PSUM") as ps_st,
            tc.tile_pool(name="ps_ot", bufs=2, space="PSUM") as ps_ot,
        ):
            # --- persistent tiles ---
            wqk_sb = pers.tile([128, NQKM, KT_C, 128], mmdt)  # resident W_qk
            kT_sb = pers.tile([128, HC * D // 128, T], bf16)  # [128, 4, 2048]
            v_aug = pers.tile([128, NKT, HC, D + 1], bf16)  # [128,16,8,65]
            wv_sb = pers.tile([128, KT_C, HC * D], mmdt)  # [128, 8, 512]
            wpj_sb = pers.tile([128, KT_P, C], bf16)  # [128, 4, 1024]
            bqk_sb = pers.tile([128, NQKM], f32)  # [128, 8]
            bv_bc = pers.tile([128, HC * D], f32)  # [128, 512]
            neg_c = pers.tile([128, 1], f32)
            bv_row = pers.tile([1, HC * D], f32)
            # combined causal masks for diagonal ST pairs: mask_a for the
            # (j0 w=512 | j1 w=384) pair, mask_b for (j2 w=256 | j3 w=128).
            mask_a = pers.tile([128, TN + 384], bf16)
            mask_b = pers.tile([128, 256 + 128], bf16)

            nc.vector.memset(v_aug[:, :, :, D : D + 1], 1.0)
            nc.vector.memset(neg_c[:], -CEXP)
            nc.vector.memset(mask_a[:], 1.0)
            nc.vector.memset(mask_b[:], 1.0)
            # each region keeps q' >= kr (q index within the causally-trimmed
            # tile vs k row within the tile)
            for mk, regions in (
                (mask_a, ((0, TN), (TN, 384))),
                (mask_b, ((0, 256), (256, 128))),
            ):
                for off, w in regions:
                    nc.gpsimd.affine_select(
                        out=mk[:, off : off + w],
                        in_=mk[:, off : off + w],
                        compare_op=mybir.AluOpType.is_ge,
                        fill=0.0,
                        base=0,
                        pattern=[[1, w]],
                        channel_multiplier=-1,
                    )
            # persistent weight loads, off the streaming (sync) DMA queue so
            # they land early without delaying xc/wqk chunk streaming
            for kh in range(4):  # quarters so the first v matmul starts early
                sl = slice(kh * (KT_C // 4), (kh + 1) * (KT_C // 4))
                nc.gpsimd.dma_start(wv_sb[:, sl, :], wv_r[:, sl, :].bitcast(mmdt))
            nc.gpsimd.dma_start(wpj_sb[:], wproj_r)
            nc.scalar.dma_start(bqk_sb[:], bqk_r)
            nc.scalar.dma_start(bv_row[:], bv[None, :])
            nc.gpsimd.partition_broadcast(bv_bc[:], bv_row[:])
            bv_hd = bv_bc[:].rearrange("p (h d) -> p h d", d=D)  # [128, 8, 64]

            for _rep in range(reps):
                for n in range(NCHUNK):
                    # ---------- phase 1: qk + v for token chunk n ----------
                    xc = xcp.tile([128, KT_C, TN], mmdt)
                    _morder = (0, 4, 1, 5, 2, 6, 3, 7)

                    def load_wm(m):
                        # stream W_qk m-tile into the persistent buffer
                        # (chunk 0 of each rep only; later chunks reuse it)
                        for kh in range(2):
                            sl = slice(kh * (KT_C // 2), (kh + 1) * (KT_C // 2))
                            nc.sync.dma_start(
                                wqk_sb[:, m, sl, :],
                                wqk_r[:, sl, m * 128 : (m + 1) * 128].bitcast(mmdt),
                            )

                    if n == 0:
                        load_wm(_morder[0])
                    for kt in range(KT_C):  # split per k-tile so matmuls start early
                        nc.sync.dma_start(
                            xc[:, kt, :],
                            xT_r[:, kt, n * TN : (n + 1) * TN].bitcast(mmdt),
                        )

                    qTc = qtcp.tile([128, HC * D // 128, TN], bf16)  # [128, 4, 512]
                    # q/k interleaved so heads unblock early (head pair hh needs
                    # only m=hh and m=4+hh)
                    for step, m in enumerate(_morder):
                        if n == 0 and step + 1 < len(_morder):
                            load_wm(_morder[step + 1])
                        ps = ps_qv.tile([128, TN], f32, tag="qv")
                        for kt in range(KT_C):
                            nc.tensor.matmul(
                                ps[:],
                                wqk_sb[:, m, kt, :],
                                xc[:, kt, :],
                                start=(kt == 0),
                                stop=(kt == KT_C - 1),
                            )
                        if m < 4:  # q features -> per-chunk qT buffer
                            dst = qTc[:, m, :]
                        else:  # k features -> persistent kT
                            dst = kT_sb[:, m - 4, n * TN : (n + 1) * TN]
                        # bias+convert on DVE: ACT is kept exp-only (it is
                        # the attention-phase bottleneck and phase 1 overlaps
                        # the previous chunk's attention)
                        nc.vector.tensor_scalar_add(
                            dst, ps[:], bqk_sb[:, m : m + 1]
                        )

                        if step == 7:
                            for mm in range(NVSUB):
                                ktg = n * NVSUB + mm
                                psv = ps_qv.tile([128, HC * D], f32, tag="qv")
                                for kt in range(KT_C):
                                    nc.tensor.matmul(
                                        psv[:],
                                        xc[:, kt, mm * 128 : (mm + 1) * 128],
                                        wv_sb[:, kt, :],
                                        start=(kt == 0),
                                        stop=(kt == KT_C - 1),
                                    )
                                nc.vector.tensor_add(
                                    out=v_aug[:, ktg, :, 0:D],
                                    in0=psv[:].rearrange("p (h d) -> p h d", d=D),
                                    in1=bv_hd,
                                )

                    # ---------- attention for q-group qg = n ----------
                    # Head pairs (2j at partitions 0:64, 2j+1 at 64:128) are
                    # processed together: their STs go to different PE
                    # row-groups and run concurrently when issued adjacently.
                    # k-tiles are processed in pairs sharing a 2-bank PSUM tile
                    # so one exp covers up to 1024 columns.  Diagonal tiles
                    # only compute causally-needed q columns (col-trim; bf16
                    # has no narrow-free-dim penalty so j3 is exactly 128).
                    qg = n
                    kt_max = NVSUB * (qg + 1)

                    def st_width(kt):
                        j = kt - NVSUB * qg
                        return TN if j < 0 else TN - 128 * j

                    def issue_pv(job, last):
                        ko, pts, ka, widths = job
                        for h2, pt in enumerate(pts):  # head 2*ko, then 2*ko+1
                            h = 2 * ko + h2
                            pso = psos[h2]
                            for kt, off, w in (
                                (ka, 0, widths[0]),
                                (ka + 1, widths[0], widths[1]),
                            ):
                                nc.tensor.matmul(
                                    pso[:, TN - w : TN],
                                    v_aug[:, kt, h, :],
                                    pt[:, off : off + w],
                                    start=(kt == 0),
                                    stop=(last and kt == ka + 1),
                                )

                    otc = otcp.tile([128, KT_P, TN], bf16)  # [128, 4, 512]
                    for ko in range(KT_P):  # head pair (2*ko, 2*ko+1)
                        psos = (
                            ps_ot.tile([D + 1, TN], f32, name="psoA", tag="pso"),
                            ps_ot.tile([D + 1, TN], f32, name="psoB", tag="pso"),
                        )
                        pending = None
                        for ka in range(0, kt_max, 2):
                            kb = ka + 1
                            wa, wb = st_width(ka), st_width(kb)
                            pss = (
                                ps_st.tile([128, 2 * TN], f32, name="pssA", tag="pss"),
                                ps_st.tile([128, 2 * TN], f32, name="pssB", tag="pss"),
                            )
                            for kt, off, w in ((ka, 0, wa), (kb, wa, wb)):
                                for h2 in range(2) if PACK_ST else (0,):
                                    pb = h2 * 64
                                    nc.tensor.matmul(
                                        pss[h2][:, off : off + w],
                                        kT_sb[pb : pb + 64, ko, kt * 128 : (kt + 1) * 128],
                                        qTc[pb : pb + 64, ko, TN - w : TN],
                                        start=True,
                                        stop=True,
                                    )
                                if not PACK_ST:
                                    nc.tensor.matmul(
                                        pss[1][:, off : off + w],
                                        kT_sb[64:128, ko, kt * 128 : (kt + 1) * 128],
                                        qTc[64:128, ko, TN - w : TN],
                                        start=True,
                                        stop=True,
                                    )
                            pts = (
                                ptp.tile([128, 2 * TN], bf16, name="ptA", tag="pt"),
                                ptp.tile([128, 2 * TN], bf16, name="ptB", tag="pt"),
                            )
                            for h2 in range(2):
                                nc.scalar.activation(
                                    pts[h2][:, 0 : wa + wb],
                                    pss[h2][:, 0 : wa + wb],
                                    Exp,
                                    bias=neg_c[:],
                                )
                            if ka >= NVSUB * qg:  # diagonal pair: combined mask
                                mk = mask_a if wa == TN else mask_b
                                for h2 in range(2):
                                    nc.vector.tensor_mul(
                                        out=pts[h2][:, 0 : wa + wb],
                                        in0=pts[h2][:, 0 : wa + wb],
                                        in1=mk[:, 0 : wa + wb],
                                    )
                            if pending is not None:
                                issue_pv(pending, last=False)
                            pending = (ko, pts, ka, (wa, wb))
                        issue_pv(pending, last=True)

                        for h2 in range(2):
                            pb = h2 * 64
                            pso = psos[h2]
                            rc = rcp.tile([1, TN], f32)
                            nc.vector.reciprocal(rc[:], pso[D : D + 1, :])
                            rcb = rcbp.tile([64, TN], f32)
                            nc.gpsimd.partition_broadcast(rcb[:], rc[:])
                            nc.vector.tensor_mul(
                                out=otc[pb : pb + 64, ko, :],
                                in0=pso[0:D, :],
                                in1=rcb[:],
                            )

                    # ---------- proj for q-group qg ----------
                    # pp shares the ST pool's 2x[128,1024] slots (tag "pss"):
                    # attention(qg) is done and attention(qg+1) starts only
                    # after phase 1 of chunk n+1, so the slots are free here.
                    for mm in range(NVSUB):
                        pp = ps_st.tile([128, 2 * TN], f32, tag="pss")
                        for nn in range(2):
                            for kt in range(KT_P):
                                nc.tensor.matmul(
                                    pp[:, nn * TN : (nn + 1) * TN],
                                    otc[:, kt, mm * 128 : (mm + 1) * 128],
                                    wpj_sb[:, kt, nn * TN : (nn + 1) * TN],
                                    start=(kt == 0),
                                    stop=(kt == KT_P - 1),
                                )
                        ob = outp.tile([128, 2 * TN], f32)
                        nc.vector.tensor_copy(ob[:], pp[:])
                        nc.sync.dma_start(
                            out[qg * TN + mm * 128 : qg * TN + (mm + 1) * 128, :],
                            ob[:],
                        )

    nc.compile()
    return nc


def _prep_inputs(x, w_qkv, b_qkv, w_proj):
    """Shard full inputs into 8 per-core input maps."""
    import ml_dtypes

    x = np.asarray(x, dtype=np.float32)
    w_qkv = np.asarray(w_qkv, dtype=np.float32)
    b_qkv = np.asarray(b_qkv, dtype=np.float32)
    w_proj = np.asarray(w_proj, dtype=np.float32)

    Wq, Wk, Wv = w_qkv[:, :C], w_qkv[:, C : 2 * C], w_qkv[:, 2 * C :]
    bq, bk, bvv = b_qkv[:C], b_qkv[C : 2 * C], b_qkv[2 * C :]
    scale = 1.0 / np.sqrt(np.float32(D))  # 0.125, exact

    in_maps = []
    for c in range(NCORES):
        b, t = divmod(c, 2)
        sl = slice(t * HC * D, (t + 1) * HC * D)
        xdt = ml_dtypes.bfloat16 if BF16 else np.float32
        in_maps.append(
            {
                "xT": np.ascontiguousarray(x[b].T.astype(xdt)),
                "wqk": np.ascontiguousarray(
                    np.concatenate([Wq[:, sl] * scale, Wk[:, sl]], axis=1).astype(xdt)
                ),
                "wv": np.ascontiguousarray(Wv[:, sl].astype(xdt)),
                "wproj": np.ascontiguousarray(
                    w_proj[sl, :].astype(ml_dtypes.bfloat16)
                ),
                "bqk": np.ascontiguousarray(
                    np.concatenate([bq[sl] * scale, bk[sl]])
                ),
                "bv": np.ascontiguousarray(bvv[sl]),
            }
        )
    return in_maps


def _run(x, w_qkv, b_qkv, w_proj, b_proj, trace=False, **trace_kwargs):
    from concourse.bass_utils import run_bass_kernel_spmd

    if "nc" not in _CACHE:
        _CACHE["nc"] = _build_program()
    nc = _CACHE["nc"]

    in_maps = _prep_inputs(x, w_qkv, b_qkv, w_proj)
    res = run_bass_kernel_spmd(
        nc, in_maps, list(range(NCORES)), trace=trace, **trace_kwargs
    )

    b_proj = np.asarray(b_proj, dtype=np.float32)
    y = np.empty((B, T, C), dtype=np.float32)
    for b in range(B):
        y[b] = res.results[2 * b]["out"] + res.results[2 * b + 1]["out"] + b_proj
    return y, res


def kernel(x, w_qkv, b_qkv, w_proj, b_proj):
    y, _ = _run(x, w_qkv, b_qkv, w_proj, b_proj, trace=False)
    return y
